# revision 1
# baseline (speedup 1.0000x reference)
"""Trainium2 Bass kernel for nn_MultiHeadAttention_79130477461654.

The reference einsum "nhqk,nhvd->nhqd" contracts k and v independently, so
out = (sum_k softmax(energy))*(sum_s v) = broadcast(sum_s v) since softmax
rows sum to 1.  With v = split_heads(x @ Wv) and the reference's direct
(n,h,q,d)->(n,s,e) reshape, the full output reduces to

    xs[n]    = sum_s x[n,s,:]                       (1024,)
    Sfull[n] = xs[n] @ Wv                           (1024,)
    WoSum    = sum_m Wo[64m+d, :]  (d=0..63)        (64, 1024)
    T[n,h,:] = Sfull[n][64h:64h+64] @ WoSum + bo    (16, 1024)
    out[n, 64h+r, :] = T[n,h,:]   for r in 0..63

numerically within ~1e-4 of the reference (softmax-row-sum rounding +
fp32r matmul rounding).  Sharding: data parallel over batch N=8, one
batch per core; Wv/Wo replicated.  All arithmetic on-device; reductions
run on the PE array chasing the DMA arrivals.
"""

import numpy as np

N, S, E, H, D = 8, 1024, 1024, 16, 64
NCORES = 8
P = 128  # partitions


def build_nc():
    import concourse.bacc as bacc
    import concourse.mybir as mybir
    from concourse.tile import TileContext

    F32 = mybir.dt.float32
    F32R = mybir.dt.float32r
    nc = bacc.Bacc("TRN2", target_bir_lowering=False, debug=False)

    xd = nc.declare_dram_parameter("x", [S, E], F32, isOutput=False)
    wvd = nc.declare_dram_parameter("Wv", [E, E], F32, isOutput=False)
    wod = nc.declare_dram_parameter("Wo", [E, E], F32, isOutput=False)
    bod = nc.declare_dram_parameter("bo128", [P, E], F32, isOutput=False)
    onesd = nc.declare_dram_parameter("ones128", [P, 1], F32, isOutput=False)
    dbld = nc.declare_dram_parameter("dblI", [P, D], F32, isOutput=False)
    outd = nc.declare_dram_parameter("out", [S, E], F32, isOutput=True)

    # two HWDGE queues: SP (sync) and ACT (scalar)
    dmae = [nc.sync, nc.scalar]

    # DRAM-side views pairing two 128-row chunks per 1 MB transfer:
    # paired(src, i)[p, c, :] = src[(2i + c)*128 + p, :]
    def paired(dram, i):
        return dram.rearrange("(i c p) e -> i p c e", p=P, c=2)[i]

    with TileContext(nc) as tc:
        with (
            tc.tile_pool(name="xin", bufs=4) as xp,
            tc.tile_pool(name="wv", bufs=4) as wvp,
            tc.tile_pool(name="wo", bufs=4) as wop,
            tc.tile_pool(name="small", bufs=1) as sp,
            tc.tile_pool(name="outsb", bufs=2) as op,
            tc.tile_pool(name="psA", bufs=1, space="PSUM") as psA,
            tc.tile_pool(name="psS", bufs=1, space="PSUM") as psS,
            tc.tile_pool(name="psF", bufs=1, space="PSUM") as psF,
            tc.tile_pool(name="psO", bufs=3, space="PSUM") as psO,
        ):
            ones_sb = sp.tile([P, 1], F32)
            dmae[0].dma_start(out=ones_sb[:], in_=onesd[:])
            dbl_sb = sp.tile([P, D], F32)
            dmae[1].dma_start(out=dbl_sb[:], in_=dbld[:])
            bo_sb = sp.tile([P, E], F32)
            dmae[1].dma_start(out=bo_sb[:], in_=bod[:])
            dbl_r = sp.tile([P, D], F32R)
            nc.vector.tensor_copy(dbl_r[:], dbl_sb[:])

            # ---- input DMAs: x, Wv, Wo as 1 MB paired transfers, 2 per queue
            #      x tiles reduce pairwise on DVE (chasing the DMAs):
            #      xacc2[p, cp*E + e] = sum_i x[(2i+cp)*128 + p, e]
            xacc2 = sp.tile([P, 2 * E], F32)
            wvt, wot = [], []
            for i in range(4):
                t = xp.tile([P, 2 * E], F32)
                dmae[i % 2].dma_start(
                    out=t[:].rearrange("p (c e) -> p c e", c=2), in_=paired(xd, i)
                )
                if i == 0:
                    nc.vector.tensor_copy(xacc2[:], t[:])
                else:
                    nc.vector.tensor_add(xacc2[:], xacc2[:], t[:])
            for i in range(4):
                t = wvp.tile([P, 2 * E], F32, tag="wvf")
                dmae[i % 2].dma_start(
                    out=t[:].rearrange("p (c e) -> p c e", c=2), in_=paired(wvd, i)
                )
                tr = wvp.tile([P, 2 * E], F32R, tag="wvr")
                nc.vector.tensor_copy(tr[:], t[:])
                wvt.append(tr)
            for i in range(4):
                t = wop.tile([P, 2 * E], F32, tag="wof")
                dmae[i % 2].dma_start(
                    out=t[:].rearrange("p (c e) -> p c e", c=2), in_=paired(wod, i)
                )
                tr = wop.tile([P, 2 * E], F32R, tag="wor")
                nc.vector.tensor_copy(tr[:], t[:])
                wot.append(tr)

            # ---- xsT[p, c] = xs[128c + p]: PE partition-reduction.
            #      Per-column groups are CONTIGUOUS (a start=True clears the
            #      whole PSUM bank's has_written, so groups sharing a bank
            #      must not interleave).
            ps_xsT = psA.tile([P, 8], F32, tag="psa")
            for c in range(8):
                for cp in range(2):
                    nc.tensor.matmul(
                        ps_xsT[:, c : c + 1],
                        xacc2[:, cp * E + c * P : cp * E + (c + 1) * P],
                        ones_sb[:],
                        start=(cp == 0),
                        stop=(cp == 1),
                    )
            xsT = sp.tile([P, 8], F32R)
            nc.vector.tensor_copy(xsT[:], ps_xsT[:])

            # ---- Sfull row (1, 1024) = xs @ Wv  (wide fp32r, chases Wv DMA)
            ps_S = psS.tile([1, E], F32, tag="pss")
            for c in range(8):
                base = (c % 2) * E
                for half in range(2):
                    sl = slice(half * 512, half * 512 + 512)
                    nc.tensor.matmul(
                        ps_S[0:1, sl],
                        xsT[:, c : c + 1],
                        wvt[c // 2][:, base + half * 512 : base + half * 512 + 512],
                        start=(c == 0),
                        stop=(c == 7),
                        skip_group_check=True,
                    )
            srow = sp.tile([1, E], F32)
            nc.vector.tensor_copy(srow[:], ps_S[:])

            # ---- sft[d, h] = Sfull[64h + d]  (N=1 fp32, base partition 0)
            ps_sft = psA.tile([D, H], F32, tag="psa")
            for h in range(H):
                nc.tensor.matmul(
                    ps_sft[:, h : h + 1],
                    srow[0:1, h * D : (h + 1) * D],
                    ones_sb[0:1, 0:1],
                    start=True,
                    stop=True,
                )
            sft = sp.tile([D, H], F32)
            nc.vector.tensor_copy(sft[:], ps_sft[:])

            # ---- rep[d, 64h + r] = sft[d, h]  (DVE free-dim broadcast, fp32r out)
            rep = sp.tile([D, H * D], F32R)
            nc.vector.tensor_copy(
                rep[:].rearrange("d (h r) -> d h r", r=D),
                sft[:, :, None].to_broadcast((D, H, D)),
            )

            # ---- WoSum[d, :] = sum_m Wo[64m + d, :]: PE fold with double
            #      identity, chasing the Wo DMAs (wide fp32r)
            ps_fold = psF.tile([D, E], F32, tag="psf")
            for i in range(4):
                for cp in range(2):
                    k = 2 * i + cp
                    for half in range(2):
                        sl = slice(half * 512, half * 512 + 512)
                        nc.tensor.matmul(
                            ps_fold[:, sl],
                            dbl_r[:],
                            wot[i][:, cp * E + half * 512 : cp * E + half * 512 + 512],
                            start=(k == 0),
                            stop=(k == 7),
                            skip_group_check=True,
                        )
            wosum = sp.tile([D, E], F32R)
            nc.vector.tensor_copy(wosum[:], ps_fold[:])

            # ---- fused T+broadcast, two 128-row blocks per 1 MB output DMA
            outr = outd.rearrange("(i c p) e -> i p c e", p=P, c=2)
            for i in range(4):
                ob = op.tile([P, 2 * E], F32)
                for c in range(2):
                    t = 2 * i + c
                    for half in range(2):
                        sl = slice(half * 512, half * 512 + 512)
                        po = psO.tile([P, 512], F32, tag="pso")
                        nc.tensor.matmul(
                            po[:],
                            rep[:, t * P : (t + 1) * P],
                            wosum[:, sl],
                            start=True,
                            stop=True,
                        )
                        # bias add fused with PSUM->SBUF move
                        nc.vector.tensor_add(
                            ob[:, c * E + half * 512 : c * E + half * 512 + 512],
                            po[:],
                            bo_sb[:, sl],
                        )
                dmae[i % 2].dma_start(
                    out=outr[i], in_=ob[:].rearrange("p (c e) -> p c e", c=2)
                )

    nc.compile()
    return nc


_NC_CACHE = None


def make_in_maps(x, Wv, Wo, bo):
    x = np.ascontiguousarray(np.asarray(x, dtype=np.float32))
    Wv = np.ascontiguousarray(np.asarray(Wv, dtype=np.float32))
    Wo = np.ascontiguousarray(np.asarray(Wo, dtype=np.float32))
    bo = np.ascontiguousarray(np.asarray(bo, dtype=np.float32))
    bo128 = np.tile(bo.reshape(1, E), (P, 1))
    ones128 = np.ones((P, 1), dtype=np.float32)
    dblI = np.zeros((P, D), dtype=np.float32)
    dblI[np.arange(P), np.arange(P) % D] = 1.0
    return [
        {
            "x": np.ascontiguousarray(x[j]),
            "Wv": Wv,
            "Wo": Wo,
            "bo128": bo128,
            "ones128": ones128,
            "dblI": dblI,
        }
        for j in range(NCORES)
    ]


def kernel(x, Wq=None, Wk=None, Wv=None, Wo=None, bo=None, **_unused):
    from concourse.bass_utils import run_bass_kernel_spmd

    global _NC_CACHE
    if _NC_CACHE is None:
        _NC_CACHE = build_nc()
    nc = _NC_CACHE

    in_maps = make_in_maps(x, Wv, Wo, bo)
    res = run_bass_kernel_spmd(nc, in_maps, core_ids=list(range(NCORES))).results
    return np.stack([res[j]["out"] for j in range(NCORES)], axis=0)



# revision 2
# speedup vs baseline: 1.0404x; 1.0404x over previous
"""Trainium2 Bass kernel for nn_MultiHeadAttention_79130477461654.

The reference einsum "nhqk,nhvd->nhqd" contracts k and v independently, so
(softmax rows sum to 1) the full output collapses to

    xs       = sum_s x[j,s,:]                      (1024,)
    Sfull    = xs @ Wv                             (1024,)
    T[h,:]   = sum_{m,d} Sfull[64h+d] Wo[64m+d,:] + bo     (16, 1024)
    out[j, 64h+r, :] = T[h,:]   for r in 0..63

Data parallel, one batch per core; all big inputs cast to bf16 on the host
(12 MB -> 6 MB per core, rel err 3.8e-3 vs the 2e-2 gate).  The Wo fold
never materializes: the T phase streams raw Wo row-chunks through the PE
against a head-replicated stationary VSrep[p,h] = Sfull[64h + p%64];
sum over chunk rows = sum over (m,d).  The bias lands as rank-1 matmuls
closing the T PSUM accumulation groups.

DMA: 48 transfers of 128 KB in consumption order x -> Wv -> Wo across the
two HWDGE queues.  The queues round-robin over queued transfers within an
~8-transfer window (the framework's DMA-semaphore recycling caps issue
lookahead), so small transfers keep completions near consumption order:
Wv lands well before Wo, VSrep is ready early, and the T phase chases the
Wo arrivals instead of serializing after them.

Output per core is the 16 distinct rows T (16, 1024); the 64x row
replication to (1024, 1024) happens on the host (pure data movement).
"""

import numpy as np
import ml_dtypes

N, S, E, H, D = 8, 1024, 1024, 16, 64
NCORES = 8
P = 128
F = 512  # transfer free width


def build_nc():
    import concourse.bacc as bacc
    import concourse.mybir as mybir
    from concourse.tile import TileContext

    F32 = mybir.dt.float32
    BF16 = mybir.dt.bfloat16
    nc = bacc.Bacc("TRN2", target_bir_lowering=False, debug=False)

    # xtb = x[j].T: e on rows so the s-reduction is a DVE free-dim reduce
    xd = nc.declare_dram_parameter("xtb", [E, S], BF16, isOutput=False)
    wvd = nc.declare_dram_parameter("wvb", [E, E], BF16, isOutput=False)
    wod = nc.declare_dram_parameter("wob", [E, E], BF16, isOutput=False)
    bod = nc.declare_dram_parameter("bob", [1, E], BF16, isOutput=False)
    dbld = nc.declare_dram_parameter("dblT", [D, P], BF16, isOutput=False)
    onesd = nc.declare_dram_parameter("ones1", [1, 1], BF16, isOutput=False)
    onesrd = nc.declare_dram_parameter("onesr", [1, H], BF16, isOutput=False)
    outd = nc.declare_dram_parameter("out", [H, E], F32, isOutput=True)

    def chunks(dram):
        return dram.rearrange("(c p) (h f) -> c h p f", p=P, f=F)

    with TileContext(nc) as tc:
        with (
            tc.tile_pool(name="xin", bufs=16) as xp,
            tc.tile_pool(name="wv", bufs=16) as wvp,
            tc.tile_pool(name="wo", bufs=16) as wop,
            tc.tile_pool(name="small", bufs=1) as sp,
            tc.tile_pool(name="psS1", bufs=1, space="PSUM") as psS1,
            tc.tile_pool(name="psS2", bufs=1, space="PSUM") as psS2,
            tc.tile_pool(name="psA", bufs=1, space="PSUM") as psA,
            tc.tile_pool(name="psR", bufs=1, space="PSUM") as psR,
            tc.tile_pool(name="psT1", bufs=1, space="PSUM") as psT1,
            tc.tile_pool(name="psT2", bufs=1, space="PSUM") as psT2,
        ):
            dmae = [nc.sync, nc.scalar]

            # small constants on the gpsimd queue
            bo_sb = sp.tile([1, E], BF16)
            nc.gpsimd.dma_start(out=bo_sb[:], in_=bod[:])
            dbl_sb = sp.tile([D, P], BF16)
            nc.gpsimd.dma_start(out=dbl_sb[:], in_=dbld[:])
            ones_sb = sp.tile([1, 1], BF16)
            nc.gpsimd.dma_start(out=ones_sb[:], in_=onesd[:])
            onesr_sb = sp.tile([1, H], BF16)
            nc.gpsimd.dma_start(out=onesr_sb[:], in_=onesrd[:])

            # ---- x: 16 transfers, DVE free-dim reduces chasing
            xsA = sp.tile([P, 8], F32)
            xsB = sp.tile([P, 8], F32)
            for c in range(8):
                for h in range(2):
                    t = xp.tile([P, F], BF16, name=f"x{c}_{h}", tag="x")
                    dmae[(2 * c + h) % 2].dma_start(out=t[:], in_=chunks(xd)[c, h])
                    dst = xsA if h == 0 else xsB
                    nc.vector.tensor_reduce(
                        dst[:, c : c + 1],
                        t[:],
                        mybir.AxisListType.X,
                        mybir.AluOpType.add,
                    )
            xsb = sp.tile([P, 8], BF16)
            nc.vector.tensor_add(xsb[:], xsA[:], xsB[:])

            # ---- Wv: Sfull row accumulation chasing Wv transfers
            ps_sf = [
                psS1.tile([1, F], F32, name="ps_sf1", tag="s1"),
                psS2.tile([1, F], F32, name="ps_sf2", tag="s2"),
            ]
            for c in range(8):
                for h in range(2):
                    t = wvp.tile([P, F], BF16, name=f"wv{c}_{h}", tag="wv")
                    dmae[(2 * c + h) % 2].dma_start(out=t[:], in_=chunks(wvd)[c, h])
                    nc.tensor.matmul(
                        ps_sf[h][:],
                        xsb[:, c : c + 1],
                        t[:],
                        start=(c == 0),
                        stop=(c == 7),
                        skip_group_check=True,
                    )
            # two independent srow halves so each half's transposes start early
            srow = [
                sp.tile([1, F], BF16, name="srow0"),
                sp.tile([1, F], BF16, name="srow1"),
            ]
            ps_vs = psA.tile([D, H], F32, tag="psa")
            vs_sb = sp.tile([D, H], BF16)
            for h in range(2):
                nc.vector.tensor_copy(srow[h][:], ps_sf[h][:])
                for hh in range(8):
                    nc.tensor.matmul(
                        ps_vs[:, 8 * h + hh : 8 * h + hh + 1],
                        srow[h][0:1, D * hh : D * (hh + 1)],
                        ones_sb[:],
                        start=True,
                        stop=True,
                    )
            nc.vector.tensor_copy(vs_sb[:], ps_vs[:])

            # VSrep[p, h] = vs[p % 64, h]
            ps_rep = psR.tile([P, H], F32, name="ps_rep", tag="psr")
            nc.tensor.matmul(ps_rep[:], dbl_sb[:], vs_sb[:], start=True, stop=True)
            vsrep = sp.tile([P, H], BF16)
            nc.vector.tensor_copy(vsrep[:], ps_rep[:])

            # ---- Wo: T accumulation chasing Wo transfers, bias closes groups
            ps_t = [
                psT1.tile([H, F], F32, name="ps_t1", tag="t1"),
                psT2.tile([H, F], F32, name="ps_t2", tag="t2"),
            ]
            for k in range(8):
                for h in range(2):
                    t = wop.tile([P, F], BF16, name=f"wo{k}_{h}", tag="wo")
                    dmae[(2 * k + h) % 2].dma_start(out=t[:], in_=chunks(wod)[k, h])
                    nc.tensor.matmul(
                        ps_t[h][:],
                        vsrep[:],
                        t[:],
                        start=(k == 0),
                        stop=False,
                        skip_group_check=True,
                    )
            out_sb = sp.tile([H, E], F32)
            for h in range(2):
                nc.tensor.matmul(
                    ps_t[h][:],
                    onesr_sb[:],
                    bo_sb[:, F * h : F * h + F],
                    start=False,
                    stop=True,
                    skip_group_check=True,
                )
                nc.vector.tensor_copy(out_sb[:, F * h : F * h + F], ps_t[h][:])
            nc.sync.dma_start(out=outd[:], in_=out_sb[:])

    nc.compile()
    return nc


_NC_CACHE = None


def make_in_maps(x, Wv, Wo, bo):
    x = np.asarray(x, dtype=np.float32)
    bo = np.asarray(bo, dtype=np.float32)
    wvb = np.ascontiguousarray(np.asarray(Wv, dtype=np.float32).astype(ml_dtypes.bfloat16))
    wob = np.ascontiguousarray(np.asarray(Wo, dtype=np.float32).astype(ml_dtypes.bfloat16))
    bob = np.ascontiguousarray(bo.reshape(1, E).astype(ml_dtypes.bfloat16))
    dblT = np.zeros((D, P), dtype=ml_dtypes.bfloat16)
    dblT[np.arange(P) % D, np.arange(P)] = 1.0
    ones1 = np.ones((1, 1), dtype=ml_dtypes.bfloat16)
    onesr = np.ones((1, H), dtype=ml_dtypes.bfloat16)
    xtb = x.transpose(0, 2, 1).astype(ml_dtypes.bfloat16)  # (N, E, S)
    return [
        {
            "xtb": np.ascontiguousarray(xtb[j]),
            "wvb": wvb,
            "wob": wob,
            "bob": bob,
            "dblT": dblT,
            "ones1": ones1,
            "onesr": onesr,
        }
        for j in range(NCORES)
    ]


def assemble(results):
    T = np.stack([results[j]["out"] for j in range(NCORES)], axis=0)  # (N, H, E)
    return np.ascontiguousarray(
        np.broadcast_to(T[:, :, None, :], (N, H, D, E)).reshape(N, S, E)
    )


def kernel(x, Wq=None, Wk=None, Wv=None, Wo=None, bo=None, **_unused):
    from concourse.bass_utils import run_bass_kernel_spmd

    global _NC_CACHE
    if _NC_CACHE is None:
        _NC_CACHE = build_nc()
    nc = _NC_CACHE

    in_maps = make_in_maps(x, Wv, Wo, bo)
    res = run_bass_kernel_spmd(nc, in_maps, core_ids=list(range(NCORES))).results
    return assemble(res)


# revision 3
# speedup vs baseline: 1.1372x; 1.0930x over previous
"""Trainium2 Bass kernel v3 for nn_MultiHeadAttention_79130477461654.

Reference einsum "nhqk,nhvd->nhqd" contracts k and v independently, so
(softmax rows sum to 1) the output collapses to

    xs[n]    = sum_s x[n,s,:]                      (1024,)
    Sfull[n] = xs[n] @ Wv                          (1024,)
    WoSum    = sum_m Wo[64m+d, :]  (d=0..63)       (64, 1024)
    T[n,h,:] = Sfull[n][64h:64h+64] @ WoSum + bo   (16, 1024)
    out[n, 64h+r, :] = T[n,h,:]   for r in 0..63

v3: data parallel, one batch per core (no cross-core traffic — NRT
collectives cost ~65us of rendezvous under this runner).  All big inputs
are cast to bf16 on the host (12 MB -> 6 MB per core; verified 3.8e-3
rel err vs the 2e-2 gate).  The Wo fold never materializes: the T-phase
streams raw Wo row-chunks through the PE against a head-replicated
stationary VSrep[p,h] = Sfull[64h + p%64], accumulating over chunks —
sum over (m,d) = sum over Wo rows.  Output per core is the 16 distinct
rows T[j] (64 KB); the 64x row replication happens on the host (pure
data movement).

Per-core DMA: x^T 2 MB + Wv 2 MB + Wo 2 MB + ~100 KB small + 64 KB out.
"""

import numpy as np
import ml_dtypes

N, S, E, H, D = 8, 1024, 1024, 16, 64
NCORES = 8
P = 128


def build_nc():
    import concourse.bacc as bacc
    import concourse.mybir as mybir
    from concourse.tile import TileContext

    F32 = mybir.dt.float32
    BF16 = mybir.dt.bfloat16
    nc = bacc.Bacc("TRN2", target_bir_lowering=False, debug=False)

    xd = nc.declare_dram_parameter("xtb", [E, S], BF16, isOutput=False)  # x[j].T
    wvd = nc.declare_dram_parameter("wvb", [E, E], BF16, isOutput=False)
    wod = nc.declare_dram_parameter("wob", [E, E], BF16, isOutput=False)
    bod = nc.declare_dram_parameter("bo16", [H, E], F32, isOutput=False)
    dbld = nc.declare_dram_parameter("dblT", [D, P], BF16, isOutput=False)
    onesd = nc.declare_dram_parameter("ones1", [1, 1], BF16, isOutput=False)
    outd = nc.declare_dram_parameter("out", [H, E], F32, isOutput=True)

    with TileContext(nc) as tc:
        with (
            tc.tile_pool(name="xin", bufs=8) as xp,
            tc.tile_pool(name="wv", bufs=8) as wvp,
            tc.tile_pool(name="wo", bufs=8) as wop,
            tc.tile_pool(name="small", bufs=1) as sp,
            tc.tile_pool(name="psS1", bufs=1, space="PSUM") as psS1,
            tc.tile_pool(name="psS2", bufs=1, space="PSUM") as psS2,
            tc.tile_pool(name="psVS", bufs=1, space="PSUM") as psVS,
            tc.tile_pool(name="psR", bufs=1, space="PSUM") as psR,
            tc.tile_pool(name="psT1", bufs=1, space="PSUM") as psT1,
            tc.tile_pool(name="psT2", bufs=1, space="PSUM") as psT2,
        ):
            dmae = [nc.sync, nc.scalar]

            # small constants on the gpsimd queue (won't delay the big loads)
            bo_sb = sp.tile([H, E], F32)
            nc.gpsimd.dma_start(out=bo_sb[:], in_=bod[:])
            dbl_sb = sp.tile([D, P], BF16)
            nc.gpsimd.dma_start(out=dbl_sb[:], in_=dbld[:])
            ones_sb = sp.tile([1, 1], BF16)
            nc.gpsimd.dma_start(out=ones_sb[:], in_=onesd[:])

            # x tiles first (xs is on the critical path), then Wv, then Wo;
            # two HWDGE queues, 256 KB per transfer, 2 KB per partition line.
            xt, wvt, wot = [], [], []
            for i in range(8):
                t = xp.tile([P, S], BF16)
                dmae[i % 2].dma_start(out=t[:], in_=xd.rearrange("(c p) s -> c p s", p=P)[i])
                xt.append(t)
            for i in range(8):
                t = wvp.tile([P, E], BF16)
                dmae[i % 2].dma_start(out=t[:], in_=wvd.rearrange("(c p) e -> c p e", p=P)[i])
                wvt.append(t)
            for i in range(8):
                t = wop.tile([P, E], BF16)
                dmae[i % 2].dma_start(out=t[:], in_=wod.rearrange("(c p) e -> c p e", p=P)[i])
                wot.append(t)

            # xsT[p, c] = sum_s x[j][s, 128c+p]: DVE free-dim reduce per tile
            xs_sb = sp.tile([P, 8], F32)
            for c in range(8):
                nc.vector.tensor_reduce(
                    xs_sb[:, c : c + 1],
                    xt[c][:],
                    mybir.AxisListType.X,
                    mybir.AluOpType.add,
                )
            xsb = sp.tile([P, 8], BF16)
            nc.vector.tensor_copy(xsb[:], xs_sb[:])

            # Sfull row (1, 1024): accumulate over e-chunks, chasing Wv DMAs
            ps_S = [
                psS1.tile([1, 512], F32, name="ps_s1"),
                psS2.tile([1, 512], F32, name="ps_s2"),
            ]
            for c in range(8):
                for half in range(2):
                    nc.tensor.matmul(
                        ps_S[half][:],
                        xsb[:, c : c + 1],
                        wvt[c][:, 512 * half : 512 * half + 512],
                        start=(c == 0),
                        stop=(c == 7),
                        skip_group_check=True,
                    )
            srow = sp.tile([1, E], BF16)
            for half in range(2):
                nc.vector.tensor_copy(
                    srow[:, 512 * half : 512 * half + 512], ps_S[half][:]
                )

            # vs[d, h] = srow[64h + d]: 16 PE column transposes
            ps_vs = psVS.tile([D, H], F32)
            for h in range(H):
                nc.tensor.matmul(
                    ps_vs[:, h : h + 1],
                    srow[0:1, D * h : D * (h + 1)],
                    ones_sb[:],
                    start=True,
                    stop=True,
                )
            vs_sb = sp.tile([D, H], BF16)
            nc.vector.tensor_copy(vs_sb[:], ps_vs[:])

            # VSrep[p, h] = vs[p % 64, h]: one PE replicate matmul
            ps_rep = psR.tile([P, H], F32)
            nc.tensor.matmul(ps_rep[:], dbl_sb[:], vs_sb[:], start=True, stop=True)
            vsrep = sp.tile([P, H], BF16)
            nc.vector.tensor_copy(vsrep[:], ps_rep[:])

            # T[h, e] = sum_k sum_p VSrep[p, h] * Wo[128k+p, e]: chases Wo DMAs
            ps_T = [
                psT1.tile([H, 512], F32, name="ps_t1"),
                psT2.tile([H, 512], F32, name="ps_t2"),
            ]
            for k in range(8):
                for half in range(2):
                    nc.tensor.matmul(
                        ps_T[half][:],
                        vsrep[:],
                        wot[k][:, 512 * half : 512 * half + 512],
                        start=(k == 0),
                        stop=(k == 7),
                        skip_group_check=True,
                    )
            out_sb = sp.tile([H, E], F32)
            for half in range(2):
                nc.vector.tensor_add(
                    out_sb[:, 512 * half : 512 * half + 512],
                    ps_T[half][:],
                    bo_sb[:, 512 * half : 512 * half + 512],
                )
            nc.sync.dma_start(out=outd[:], in_=out_sb[:])

    nc.compile()
    return nc


_NC_CACHE = None


def make_in_maps(x, Wv, Wo, bo):
    x = np.asarray(x, dtype=np.float32)
    bo = np.asarray(bo, dtype=np.float32)
    wvb = np.ascontiguousarray(np.asarray(Wv, dtype=np.float32).astype(ml_dtypes.bfloat16))
    wob = np.ascontiguousarray(np.asarray(Wo, dtype=np.float32).astype(ml_dtypes.bfloat16))
    bo16 = np.ascontiguousarray(np.tile(bo.reshape(1, E), (H, 1)))
    dblT = np.zeros((D, P), dtype=ml_dtypes.bfloat16)
    dblT[np.arange(P) % D, np.arange(P)] = 1.0
    ones1 = np.ones((1, 1), dtype=ml_dtypes.bfloat16)
    xtb = x.transpose(0, 2, 1).astype(ml_dtypes.bfloat16)  # (N, E, S)
    return [
        {
            "xtb": np.ascontiguousarray(xtb[j]),
            "wvb": wvb,
            "wob": wob,
            "bo16": bo16,
            "dblT": dblT,
            "ones1": ones1,
        }
        for j in range(NCORES)
    ]


def assemble(results):
    T = np.stack([results[j]["out"] for j in range(NCORES)], axis=0)  # (N, H, E)
    return np.ascontiguousarray(
        np.broadcast_to(T[:, :, None, :], (N, H, D, E)).reshape(N, S, E)
    )


def kernel(x, Wq=None, Wk=None, Wv=None, Wo=None, bo=None, **_unused):
    from concourse.bass_utils import run_bass_kernel_spmd

    global _NC_CACHE
    if _NC_CACHE is None:
        _NC_CACHE = build_nc()
    nc = _NC_CACHE

    in_maps = make_in_maps(x, Wv, Wo, bo)
    res = run_bass_kernel_spmd(nc, in_maps, core_ids=list(range(NCORES))).results
    return assemble(res)


# revision 4
# speedup vs baseline: 1.1984x; 1.0539x over previous
"""Trainium2 Bass kernel v7 for nn_MultiHeadAttention_79130477461654.

Math (reference einsum contracts k and v independently; softmax rows sum to 1):

    xs       = sum_s x[j,s,:]                      (1024,)
    Sfull    = xs @ Wv                             (1024,)
    T[h,:]   = sum_{m,d} Sfull[64h+d] Wo[64m+d,:] + bo     (16, 1024)
    out[j, 64h+r, :] = T[h,:]   for r in 0..63

v7 = v3 with a pipelined epilogue.  v3's post-DMA tail (srow copies ->
16 transposes -> replicate -> T -> bias -> one out DMA) ran 7.6-11 us
fully serialized after the last Wv chunk landed.  Here:
  - srow is two half tiles; each half's copy + 8 transposes start as soon
    as that half's Sfull PSUM group closes.
  - The T phase runs half-OUTER (all 8 chunks for e' 0..511, then all 8
    for e' 512..1023), so half 0's bias-add + output DMA overlap half 1's
    matmuls.  Two independent out DMAs on the two queues.
Everything else (data parallel one batch/core, bf16 inputs, DVE reduces
chasing x, VSrep trick, 16 distinct output rows + host row replication)
is unchanged from v3.
"""

import numpy as np
import ml_dtypes

N, S, E, H, D = 8, 1024, 1024, 16, 64
NCORES = 8
P = 128


def build_nc():
    import concourse.bacc as bacc
    import concourse.mybir as mybir
    from concourse.tile import TileContext

    F32 = mybir.dt.float32
    BF16 = mybir.dt.bfloat16
    nc = bacc.Bacc("TRN2", target_bir_lowering=False, debug=False)

    xd = nc.declare_dram_parameter("xtb", [E, S], BF16, isOutput=False)  # x[j].T
    wvd = nc.declare_dram_parameter("wvb", [E, E], BF16, isOutput=False)
    wod = nc.declare_dram_parameter("wob", [E, E], BF16, isOutput=False)
    bod = nc.declare_dram_parameter("bo16", [H, E], F32, isOutput=False)
    dbld = nc.declare_dram_parameter("dblT", [D, P], BF16, isOutput=False)
    onesd = nc.declare_dram_parameter("ones1", [1, 1], BF16, isOutput=False)
    outd = [
        nc.declare_dram_parameter("outA", [H, 512], F32, isOutput=True),
        nc.declare_dram_parameter("outB", [H, 512], F32, isOutput=True),
    ]

    with TileContext(nc) as tc:
        with (
            tc.tile_pool(name="xin", bufs=8) as xp,
            tc.tile_pool(name="wv", bufs=8) as wvp,
            tc.tile_pool(name="wo", bufs=8) as wop,
            tc.tile_pool(name="small", bufs=1) as sp,
            tc.tile_pool(name="psS1", bufs=1, space="PSUM") as psS1,
            tc.tile_pool(name="psS2", bufs=1, space="PSUM") as psS2,
            tc.tile_pool(name="psA", bufs=1, space="PSUM") as psA,
            tc.tile_pool(name="psR", bufs=1, space="PSUM") as psR,
            tc.tile_pool(name="psT1", bufs=1, space="PSUM") as psT1,
            tc.tile_pool(name="psT2", bufs=1, space="PSUM") as psT2,
        ):
            dmae = [nc.sync, nc.scalar]

            # small constants on the gpsimd queue
            bo_sb = sp.tile([H, E], F32)
            nc.gpsimd.dma_start(out=bo_sb[:], in_=bod[:])
            dbl_sb = sp.tile([D, P], BF16)
            nc.gpsimd.dma_start(out=dbl_sb[:], in_=dbld[:])
            ones_sb = sp.tile([1, 1], BF16)
            nc.gpsimd.dma_start(out=ones_sb[:], in_=onesd[:])

            # big loads: x row-chunks of x^T, then Wv, then Wo (256 KB each)
            xt, wvt, wot = [], [], []
            for i in range(8):
                t = xp.tile([P, S], BF16, name=f"x{i}", tag="x")
                dmae[i % 2].dma_start(
                    out=t[:], in_=xd.rearrange("(c p) s -> c p s", p=P)[i]
                )
                xt.append(t)
            for i in range(8):
                t = wvp.tile([P, E], BF16, name=f"wv{i}", tag="wv")
                dmae[i % 2].dma_start(
                    out=t[:], in_=wvd.rearrange("(c p) e -> c p e", p=P)[i]
                )
                wvt.append(t)
            for i in range(8):
                t = wop.tile([P, E], BF16, name=f"wo{i}", tag="wo")
                dmae[i % 2].dma_start(
                    out=t[:], in_=wod.rearrange("(c p) e -> c p e", p=P)[i]
                )
                wot.append(t)

            # xsT[p, c] = xs[128c + p]: DVE free-dim reduces chasing x DMAs
            xs_sb = sp.tile([P, 8], F32)
            for c in range(8):
                nc.vector.tensor_reduce(
                    xs_sb[:, c : c + 1],
                    xt[c][:],
                    mybir.AxisListType.X,
                    mybir.AluOpType.add,
                )
            xsb = sp.tile([P, 8], BF16)
            nc.vector.tensor_copy(xsb[:], xs_sb[:])

            # Sfull row (1, 1024): accumulate over e-chunks, chasing Wv DMAs
            ps_sf = [
                psS1.tile([1, 512], F32, name="ps_sf1", tag="s1"),
                psS2.tile([1, 512], F32, name="ps_sf2", tag="s2"),
            ]
            for c in range(8):
                for half in range(2):
                    nc.tensor.matmul(
                        ps_sf[half][:],
                        xsb[:, c : c + 1],
                        wvt[c][:, 512 * half : 512 * half + 512],
                        start=(c == 0),
                        stop=(c == 7),
                        skip_group_check=True,
                    )

            # per-half: srow copy then 8 column transposes — half 0's chain
            # starts while half 1's group may still be accumulating
            srow = [
                sp.tile([1, 512], BF16, name="srow0"),
                sp.tile([1, 512], BF16, name="srow1"),
            ]
            ps_vs = psA.tile([D, H], F32, tag="psa")
            for half in range(2):
                nc.vector.tensor_copy(srow[half][:], ps_sf[half][:])
                for hh in range(8):
                    nc.tensor.matmul(
                        ps_vs[:, 8 * half + hh : 8 * half + hh + 1],
                        srow[half][0:1, D * hh : D * (hh + 1)],
                        ones_sb[:],
                        start=True,
                        stop=True,
                    )
            vs_sb = sp.tile([D, H], BF16)
            nc.vector.tensor_copy(vs_sb[:], ps_vs[:])

            # VSrep[p, h] = vs[p % 64, h]
            ps_rep = psR.tile([P, H], F32, name="ps_rep", tag="psr")
            nc.tensor.matmul(ps_rep[:], dbl_sb[:], vs_sb[:], start=True, stop=True)
            vsrep = sp.tile([P, H], BF16)
            nc.vector.tensor_copy(vsrep[:], ps_rep[:])

            # T phase, half-OUTER: finish e' 0..511 completely, then its
            # bias-add + out DMA overlap the e' 512..1023 matmuls.
            ps_t = [
                psT1.tile([H, 512], F32, name="ps_t1", tag="t1"),
                psT2.tile([H, 512], F32, name="ps_t2", tag="t2"),
            ]
            out_sb = [
                sp.tile([H, 512], F32, name="outsb0"),
                sp.tile([H, 512], F32, name="outsb1"),
            ]
            for half in range(2):
                for k in range(8):
                    nc.tensor.matmul(
                        ps_t[half][:],
                        vsrep[:],
                        wot[k][:, 512 * half : 512 * half + 512],
                        start=(k == 0),
                        stop=(k == 7),
                        skip_group_check=True,
                    )
                nc.vector.tensor_add(
                    out_sb[half][:],
                    ps_t[half][:],
                    bo_sb[:, 512 * half : 512 * half + 512],
                )
                dmae[1 - half].dma_start(out=outd[half][:], in_=out_sb[half][:])

    nc.compile()
    return nc


_NC_CACHE = None


def make_in_maps(x, Wv, Wo, bo):
    x = np.asarray(x, dtype=np.float32)
    bo = np.asarray(bo, dtype=np.float32)
    wvb = np.ascontiguousarray(np.asarray(Wv, dtype=np.float32).astype(ml_dtypes.bfloat16))
    wob = np.ascontiguousarray(np.asarray(Wo, dtype=np.float32).astype(ml_dtypes.bfloat16))
    bo16 = np.ascontiguousarray(np.tile(bo.reshape(1, E), (H, 1)))
    dblT = np.zeros((D, P), dtype=ml_dtypes.bfloat16)
    dblT[np.arange(P) % D, np.arange(P)] = 1.0
    ones1 = np.ones((1, 1), dtype=ml_dtypes.bfloat16)
    xtb = x.transpose(0, 2, 1).astype(ml_dtypes.bfloat16)  # (N, E, S)
    return [
        {
            "xtb": np.ascontiguousarray(xtb[j]),
            "wvb": wvb,
            "wob": wob,
            "bo16": bo16,
            "dblT": dblT,
            "ones1": ones1,
        }
        for j in range(NCORES)
    ]


def assemble(results):
    T = np.empty((N, H, E), dtype=np.float32)
    for j in range(NCORES):
        T[j, :, 0:512] = results[j]["outA"]
        T[j, :, 512:1024] = results[j]["outB"]
    return np.ascontiguousarray(
        np.broadcast_to(T[:, :, None, :], (N, H, D, E)).reshape(N, S, E)
    )


def kernel(x, Wq=None, Wk=None, Wv=None, Wo=None, bo=None, **_unused):
    from concourse.bass_utils import run_bass_kernel_spmd

    global _NC_CACHE
    if _NC_CACHE is None:
        _NC_CACHE = build_nc()
    nc = _NC_CACHE

    in_maps = make_in_maps(x, Wv, Wo, bo)
    res = run_bass_kernel_spmd(nc, in_maps, core_ids=list(range(NCORES))).results
    return assemble(res)
